# revision 18
# baseline (speedup 1.0000x reference)
"""Trainium2 Bass kernel for nn_DeformableBlock (offset conv -> deformable 3x3
conv via bilinear sampling -> GroupNorm(16) -> LeakyReLU(0.2)).

Sharding: 8 cores = (batch 4) x (H halves 2). Each core computes its
(batch, 64-row half) with a 2-row sampling halo.

Device algorithm (per core), exact for offsets |d|<=1 (clamped otherwise):
  bilinear sample at p+(ky,kx)+(dy,dx) == sum_{a,b in 3x3} wy_a(dy)*wx_b(dx)
  * x[p+(ky+a,kx+b)], with wy_{-1}=min(relu(-d),1), wy_0=1-|clamp(d)|,
  wy_1=min(relu(d),1). So
    out[o,p] = sum_{k,a,b} sum_c w_def[o,c,k] * c_{k,ab}[p] * xs_{k,ab}[c,p]
  i.e. a K=(64c x 81 terms) matmul whose rhs rows are coefficient-modulated
  shifted images, built by DVE muls with DMA-broadcast fp8 coefficients.
  Terms are packed in pairs (shift delta = +1 row) onto 128 partitions.

Single phase: GroupNorm stats are AllReduce'd across the 2-core half pairs,
normalize+LeakyReLU run on device (one Lrelu activation). The few pixels
with |d|>1 (34 for this input scale) are patched exactly on host by
inverting the LeakyReLU with the exported per-channel scale A.
"""

LAST_EXEC_NS = None
LAST_RES1 = None
LAST_RES2 = None

import sys
import types

import numpy as np

# The trimmed container lacks antenv.axon_hooks; BASS_TRACE=1 would crash
# run_bass_kernel_spmd on import. Shim it, wiring the ctypes NTFF hook from
# trn_agent_boot when available so tracing works; otherwise "no hook".
try:
    from antenv import axon_hooks as _ah  # noqa: F401
except ImportError:
    def _mk_hook():
        try:
            from trn_agent_boot.trn_boot import _ntff_profile_via_ctypes
            return _ntff_profile_via_ctypes("/opt/axon/libaxon_pjrt.so")
        except Exception:
            return None

    _m = types.ModuleType("antenv.axon_hooks")
    _m.get_axon_ntff_profile_hook = _mk_hook
    sys.modules["antenv.axon_hooks"] = _m

import concourse.bacc as bacc
import concourse.tile as tile
from concourse import mybir
from concourse.bass_utils import run_bass_kernel_spmd

F32 = mybir.dt.float32
F16 = mybir.dt.float16
F8 = mybir.dt.float16  # fp8 fails the 2e-2 gate; broadcast is packet-bound anyway

B, C, O, H, W = 4, 64, 64, 128, 128
HH = 64          # rows per half
RT = HH + 4      # 68 rows incl 2-halo each side
WP = W + 4       # 132 padded cols
NPX = HH * W     # 8192 output pixels per core
G = 16
GSZ = O // G
EPS = 1e-5
NEG = 0.2
GN_N = GSZ * H * W  # group-norm reduction count per (batch, group)

# ---------------------------------------------------------------- term table


def _build_terms():
    """81 (k, ai, bi) terms; pack into pairs with shift delta (+1 row, 0)."""
    terms = []
    for k in range(9):
        for ai in range(3):
            for bi in range(3):
                sy = (k // 3 - 1) + (ai - 1)
                sx = (k % 3 - 1) + (bi - 1)
                sign = (-1 if ai == 0 else 1) * (-1 if bi == 0 else 1)
                terms.append((k, ai, bi, sy, sx, sign))
    buckets = {}
    for t in terms:
        buckets.setdefault((t[3], t[4]), []).append(t)
    pairs, used = [], set()
    for sy in range(-2, 2):
        for sx in range(-2, 3):
            lo = buckets.get((sy, sx), [])
            hi = buckets.get((sy + 1, sx), [])
            while lo and hi:
                t1, t2 = lo.pop(), hi.pop()
                if id(t1) in used or id(t2) in used:
                    continue
                used.add(id(t1)), used.add(id(t2))
                pairs.append((t1, t2))
    singles = [t for t in terms if id(t) not in used]
    return pairs, singles


PAIRS, SINGLES = _build_terms()
NT = len(PAIRS) + len(SINGLES)

# ------------------------------------------------------------- device build


def build_kernel():
    nc = bacc.Bacc("TRN2", target_bir_lowering=False, debug=False, num_devices=8)
    xh = nc.dram_tensor("xh", [C, RT, WP], F16, kind="ExternalInput")
    lhs_off = nc.dram_tensor("lhs_off", [3, 128, 41], F16, kind="ExternalInput")
    lhs_off_s = nc.dram_tensor("lhs_off_s", [3, 64, 41], F16, kind="ExternalInput")
    bias_off = nc.dram_tensor("bias_off", [41, 1], F32, kind="ExternalInput")
    lhs_main = nc.dram_tensor("lhs_main", [NT, 128, O], F16, kind="ExternalInput")
    bias_def = nc.dram_tensor("bias_def", [O, 1], F32, kind="ExternalInput")
    gn_wb = nc.dram_tensor("gn_wb", [O, 2], F32, kind="ExternalInput")
    gsum = nc.dram_tensor("gsum", [O, O], F32, kind="ExternalInput")

    y_out = nc.dram_tensor("y", [O, NPX], F16, kind="ExternalOutput")
    dy_out = nc.dram_tensor("dy_out", [128, 2048], F32, kind="ExternalOutput")
    dx_out = nc.dram_tensor("dx_out", [128, 2048], F32, kind="ExternalOutput")
    ab_out = nc.dram_tensor("ab_out", [O, 2], F32, kind="ExternalOutput")

    NW = RT * WP  # 8976

    MIN, MAX = mybir.AluOpType.min, mybir.AluOpType.max
    ADD, SUB, MUL = (
        mybir.AluOpType.add,
        mybir.AluOpType.subtract,
        mybir.AluOpType.mult,
    )
    IDENT = mybir.ActivationFunctionType.Identity

    with tile.TileContext(nc) as tc:
        with (
            tc.tile_pool(name="x16p", bufs=1) as x16p,
            tc.tile_pool(name="small", bufs=1) as small,
            tc.tile_pool(name="dram", bufs=1, space="DRAM") as dpool,
            tc.tile_pool(name="bcast", bufs=3) as bpool,
            tc.tile_pool(name="mt", bufs=3) as mpool,
        ):
            # ---- weights to SBUF
            lw = small.tile([128, NT * O], F16)
            nc.sync.dma_start(
                lw[:].rearrange("p (t o) -> p t o", t=NT),
                lhs_main.ap().transpose([1, 0, 2]),
            )
            lo = small.tile([128, 3 * 41], F16)
            nc.sync.dma_start(
                lo[:].rearrange("p (t o) -> p t o", t=3),
                lhs_off.ap().transpose([1, 0, 2]),
            )
            los = small.tile([64, 3 * 41], F16)
            nc.sync.dma_start(
                los[:].rearrange("p (t o) -> p t o", t=3),
                lhs_off_s.ap().transpose([1, 0, 2]),
            )
            bo = small.tile([41, 1], F32)
            nc.sync.dma_start(bo[:], bias_off[:, :])
            bd = small.tile([O, 1], F32)
            nc.sync.dma_start(bd[:], bias_def[:, :])
            gwb = small.tile([O, 2], F32)
            nc.sync.dma_start(gwb[:], gn_wb[:, :])
            gsm = small.tile([O, O], F32)
            nc.sync.dma_start(gsm[:], gsum[:, :])

            # ---- x (two row-shifted partition halves), fp16, plus odd-col copy
            x16e = x16p.tile([128, NW], F16)
            x16o = x16p.tile([128, NW], F16)
            nc.sync.dma_start(x16e[0:64, :], xh[:, :, :])
            nc.sync.dma_start(x16e[64:128, 0 : (RT - 1) * WP], xh[:, 1:RT, :])
            nc.vector.memset(x16e[64:128, (RT - 1) * WP : NW], 0.0)
            nc.vector.tensor_copy(x16o[:, 0 : NW - 1], x16e[:, 1:NW])
            nc.vector.memset(x16o[:, NW - 1 : NW], 0.0)

            # cdr81[ab*9+k, q*2048 + f]: coefficient maps, fp8, in DRAM
            # (SBUF DMA sources cannot have the stride-0 partition dim that
            # partition_broadcast needs, so the broadcast reads DRAM rows)
            cdr81 = dpool.tile([81, NPX], F8)

            with (
                tc.tile_pool(name="work", bufs=1) as work,
                tc.tile_pool(name="cabp", bufs=2) as cabp,
            ):
                # ---- offset conv (fp16, M=18) -> dy_sb/dx_sb [128, 2048]:
                # partition 32*q + k (q = pixel quarter, k = tap), free =
                # pixel-within-quarter; partitions 9-31 of each group unused.
                dy_sb = work.tile([128, 2048], F32, tag="dy")
                dx_sb = work.tile([128, 2048], F32, tag="dx")
                nc.vector.memset(dy_sb[:], 0.0)
                nc.vector.memset(dx_sb[:], 0.0)
                x16ev = x16e[:].rearrange("p (r w) -> p r w", w=WP)
                with tc.tile_pool(name="psoff", bufs=2, space="PSUM") as psoff:
                    for ch in range(16):  # 512-px chunks: rows 4ch..4ch+3
                        j0 = 4 * ch
                        q, fo = ch // 4, (ch % 4) * 512
                        ps = psoff.tile([41, 512], F32, tag="psoff")
                        for p in range(3):  # pairs (ky=-1, ky=0), kx = p-1
                            rhs = x16ev[:, j0 + 1 : j0 + 5, 1 + p : 1 + p + W]
                            nc.tensor.matmul(
                                ps[:],
                                lo[:, p * 41 : (p + 1) * 41],
                                rhs,
                                start=(p == 0),
                                stop=False,
                            )
                        for p in range(3):  # singles ky=+1
                            rhs = x16ev[0:64, j0 + 3 : j0 + 7, 1 + p : 1 + p + W]
                            nc.tensor.matmul(
                                ps[:],
                                los[:, p * 41 : (p + 1) * 41],
                                rhs,
                                start=False,
                                stop=(p == 2),
                            )
                        nc.scalar.activation(
                            dy_sb[32 * q : 32 * q + 9, fo : fo + 512],
                            ps[0:9, :], IDENT, bias=bo[0:9],
                        )
                        nc.scalar.activation(
                            dx_sb[32 * q : 32 * q + 9, fo : fo + 512],
                            ps[32:41, :], IDENT, bias=bo[32:41],
                        )
                nc.sync.dma_start(dy_out[:, :], dy_sb[:])
                nc.sync.dma_start(dx_out[:, :], dx_sb[:])

                # ---- coefficient factors (negated where noted) and products
                wfac = {}
                for nm, src in (("y", dy_sb), ("x", dx_sb)):
                    nm1 = work.tile([128, 2048], F16, tag=f"nm1{nm}")
                    np1 = work.tile([128, 2048], F16, tag=f"np1{nm}")
                    w0 = work.tile([128, 2048], F16, tag=f"w0{nm}")
                    # nm1 = max(min(d,0), -1) = -min(relu(-d), 1)
                    nc.vector.tensor_scalar(nm1[:], src[:], 0.0, -1.0, MIN, MAX)
                    # np1 = min(max(d,0), 1)
                    nc.vector.tensor_scalar(np1[:], src[:], 0.0, 1.0, MAX, MIN)
                    # w0 = 1 + (nm1 - np1) = 1 - |clamp(d)|
                    nc.vector.tensor_tensor(w0[:], nm1[:], np1[:], SUB)
                    nc.vector.tensor_scalar(w0[:], w0[:], 1.0, None, ADD)
                    wfac[nm] = [nm1, w0, np1]
                ab_order = []
                for pr in PAIRS + SINGLES:
                    for t in (pr if isinstance(pr, tuple) and isinstance(pr[0], tuple) else (pr,)):
                        ab = t[1] * 3 + t[2]
                        if ab not in ab_order:
                            ab_order.append(ab)
                for ab in ab_order:
                    ai, bi = ab // 3, ab % 3
                    if True:
                        cab = cabp.tile([128, 2048], F8, tag="cab")
                        nc.vector.tensor_tensor(
                            cab[:], wfac["y"][ai][:], wfac["x"][bi][:], MUL
                        )
                        # reshuffle (q,k)-partition layout into [9, (q f)] rows
                        r0 = (ai * 3 + bi) * 9
                        for q in range(4):
                            nc.sync.dma_start(
                                cdr81[r0 : r0 + 9, q * 2048 : (q + 1) * 2048],
                                cab[32 * q : 32 * q + 9, :],
                            )

            # ---- main modulated matmul, two 4096-px halves
            osb = small.tile([O, NPX], F32)
            st = small.tile([O, 4], F32)
            x16ev2 = x16e[:].rearrange("p (r w) -> p r w", w=WP)
            x16ov2 = x16o[:].rearrange("p (r w) -> p r w", w=WP)

            def src_view(sy, sx, j0):
                r = j0 + 2 + sy
                cs = 2 + sx
                if cs % 2 == 0:
                    return x16ev2[:, r : r + 32, cs : cs + W]
                return x16ov2[:, r : r + 32, cs - 1 : cs - 1 + W]

            with tc.tile_pool(name="psout", bufs=1, space="PSUM") as psout:
                for hf2 in range(2):
                    j0 = 32 * hf2
                    fo = hf2 * 4096
                    ps = psout.tile([O, 4096], F32)
                    for ti, pr in enumerate(PAIRS + SINGLES):
                        if ti < len(PAIRS):
                            t1, t2 = pr
                            kparts = 128
                        else:
                            t1, t2 = pr, None
                            kparts = 64
                        bt = bpool.tile([128, 4096], F8, tag="bt")
                        k1, a1, b1, sy1, sx1, _ = t1
                        r1 = (a1 * 3 + b1) * 9 + k1
                        nc.sync.dma_start(
                            bt[0:64, :],
                            cdr81[r1 : r1 + 1, fo : fo + 4096]
                            .partition_broadcast(64),
                        )
                        if t2 is not None:
                            k2, a2, b2, _, _, _ = t2
                            r2 = (a2 * 3 + b2) * 9 + k2
                            nc.sync.dma_start(
                                bt[64:128, :],
                                cdr81[r2 : r2 + 1, fo : fo + 4096]
                                .partition_broadcast(64),
                            )
                        mt = mpool.tile([128, 4096], F16, tag="mt")
                        nc.vector.tensor_tensor(
                            mt[0:kparts, :].rearrange("p (r w) -> p r w", w=W),
                            bt[0:kparts, :].rearrange("p (r w) -> p r w", w=W),
                            src_view(sy1, sx1, j0)[0:kparts],
                            MUL,
                        )
                        for q in range(8):
                            nc.tensor.matmul(
                                ps[:, q * 512 : (q + 1) * 512],
                                lw[0:kparts, ti * O : (ti + 1) * O],
                                mt[0:kparts, q * 512 : (q + 1) * 512],
                                start=(ti == 0),
                                stop=(ti == NT - 1),
                            )
                    sl = slice(fo, fo + 4096)
                    nc.scalar.activation(
                        osb[:, sl], ps[:], IDENT, bias=bd[:],
                        accum_out=st[:, hf2 : hf2 + 1],
                    )
                    sq = mpool.tile([128, 4096], F16, tag="mt")
                    nc.scalar.activation(
                        sq[0:64, :], osb[:, sl],
                        mybir.ActivationFunctionType.Square,
                        accum_out=st[:, 2 + hf2 : 3 + hf2],
                    )

            # ---- GroupNorm: combine halves, AllReduce across the core pair
            st2 = small.tile([O, 2], F32)
            nc.vector.tensor_tensor(st2[:, 0:1], st[:, 0:1], st[:, 1:2], ADD)
            nc.vector.tensor_tensor(st2[:, 1:2], st[:, 2:3], st[:, 3:4], ADD)
            stin = dpool.tile([O, 2], F32)
            stout = dpool.tile([O, 2], F32)
            nc.sync.dma_start(stin[:, :], st2[:])
            nc.gpsimd.collective_compute(
                "AllReduce",
                mybir.AluOpType.add,
                replica_groups=[[0, 1], [2, 3], [4, 5], [6, 7]],
                ins=[stin[:, :]],
                outs=[stout[:, :]],
            )
            stt = small.tile([O, 2], F32)
            nc.sync.dma_start(stt[:], stout[:, :])

            with tc.tile_pool(name="psgn", bufs=1, space="PSUM") as psgn:
                gsp = psgn.tile([O, 2], F32)
                nc.tensor.matmul(gsp[:], gsm[:], stt[:], start=True, stop=True)
                mus = small.tile([O, 2], F32)
                nc.vector.tensor_scalar_mul(mus[:], gsp[:], 1.0 / GN_N)
            m2 = small.tile([O, 1], F32)
            nc.vector.tensor_tensor(m2[:], mus[:, 0:1], mus[:, 0:1], MUL)
            var = small.tile([O, 1], F32)
            nc.vector.tensor_tensor(var[:], mus[:, 1:2], m2[:], SUB)
            nc.vector.tensor_scalar(var[:], var[:], EPS, None, ADD)
            sd = small.tile([O, 1], F32)
            nc.scalar.activation(sd[:], var[:], mybir.ActivationFunctionType.Sqrt)
            rstd = small.tile([O, 1], F32)
            nc.vector.reciprocal(rstd[:], sd[:])
            ab = small.tile([O, 2], F32)
            # A = rstd * gn_w ; Bc = gn_b - mu * A
            nc.vector.tensor_tensor(ab[:, 0:1], rstd[:], gwb[:, 0:1], MUL)
            tmp = small.tile([O, 1], F32)
            nc.vector.tensor_tensor(tmp[:], mus[:, 0:1], ab[:, 0:1], MUL)
            nc.vector.tensor_tensor(ab[:, 1:2], gwb[:, 1:2], tmp[:], SUB)
            nc.sync.dma_start(ab_out[:, :], ab[:])

            # ---- y = LeakyRelu(A*pre + B), fp16 out (explicit: the Lrelu
            # activation's alpha immediate is ignored by the act table)
            zsb = small.tile([O, NPX], F16)
            nc.scalar.activation(
                zsb[:], osb[:], IDENT, bias=ab[:, 1:2], scale=ab[:, 0:1],
            )
            ysb = small.tile([O, NPX], F16)
            nc.vector.tensor_scalar_mul(ysb[:], zsb[:], NEG)
            nc.vector.tensor_tensor(ysb[:], zsb[:], ysb[:], mybir.AluOpType.max)
            nc.sync.dma_start(y_out[:, :], ysb[:])
    nc.compile()
    return nc


# ----------------------------------------------------------------- host side


def _host_inputs(x, w_off, b_off, w_def, b_def, gn_w, gn_b):
    """Per-core input maps."""
    # offset-conv output channel order: cols 0-8 = dy taps, 9-17 = dx taps
    perm = [2 * k for k in range(9)] + [2 * k + 1 for k in range(9)]
    lhs_off = np.zeros((3, 128, 41), np.float16)
    lhs_off_s = np.zeros((3, 64, 41), np.float16)
    for p in range(3):
        lhs_off[p, 0:64, 0:9] = w_off[perm[0:9], :, 0, p].T
        lhs_off[p, 64:128, 0:9] = w_off[perm[0:9], :, 1, p].T
        lhs_off[p, 0:64, 32:41] = w_off[perm[9:18], :, 0, p].T
        lhs_off[p, 64:128, 32:41] = w_off[perm[9:18], :, 1, p].T
        lhs_off_s[p, :, 0:9] = w_off[perm[0:9], :, 2, p].T
        lhs_off_s[p, :, 32:41] = w_off[perm[9:18], :, 2, p].T
    lhs_main = np.zeros((NT, 128, O), np.float16)
    for ti, pr in enumerate(PAIRS + SINGLES):
        if ti < len(PAIRS):
            t1, t2 = pr
        else:
            t1, t2 = pr, None
        k1, a1, b1, _, _, s1 = t1
        lhs_main[ti, 0:64] = (s1 * w_def[:, :, k1 // 3, k1 % 3]).T
        if t2 is not None:
            k2, a2, b2, _, _, s2 = t2
            lhs_main[ti, 64:128] = (s2 * w_def[:, :, k2 // 3, k2 % 3]).T
    bo2 = b_off.reshape(9, 2)
    bias18 = np.zeros((41, 1), np.float32)
    bias18[0:9, 0] = bo2[:, 0]
    bias18[32:41, 0] = bo2[:, 1]
    gsmat = np.zeros((O, O), np.float32)
    for g in range(G):
        gsmat[g * GSZ : (g + 1) * GSZ, g * GSZ : (g + 1) * GSZ] = 1.0
    shared = {
        "lhs_off": lhs_off,
        "lhs_off_s": lhs_off_s,
        "bias_off": bias18,
        "lhs_main": lhs_main,
        "bias_def": b_def.reshape(O, 1).astype(np.float32),
        "gn_wb": np.stack([gn_w, gn_b], 1).astype(np.float32),
        "gsum": gsmat,
    }
    maps = []
    for core in range(8):
        bb, hf = core // 2, core % 2
        r0 = HH * hf
        xhd = np.zeros((C, RT, WP), np.float16)
        lo = max(0, r0 - 2)
        hi = min(H, r0 + HH + 2)
        xhd[:, lo - (r0 - 2) : hi - (r0 - 2), 2 : 2 + W] = x[bb, :, lo:hi, :]
        maps.append({"xh": xhd, **shared})
    return maps


def _bilin(xb, k, h, w, dy, dx):
    ky, kx = k // 3 - 1, k % 3 - 1
    py, px = h + ky + dy, w + kx + dx
    y0, x0 = np.floor(py), np.floor(px)
    wy, wx = np.float32(py - y0), np.float32(px - x0)
    acc = np.zeros(xb.shape[0], np.float32)
    for u, wu in ((0, 1 - wy), (1, wy)):
        for v, wv in ((0, 1 - wx), (1, wx)):
            yc, xc = int(y0) + u, int(x0) + v
            if 0 <= yc < H and 0 <= xc < W:
                acc += np.float32(wu * wv) * xb[:, yc, xc]
    return acc


def kernel(x, w_off, b_off, w_def, b_def, gn_w, gn_b):
    x = np.asarray(x, np.float32)
    w_off = np.asarray(w_off, np.float32)
    b_off = np.asarray(b_off, np.float32)
    w_def = np.asarray(w_def, np.float32)
    b_def = np.asarray(b_def, np.float32)
    gn_w = np.asarray(gn_w, np.float32)
    gn_b = np.asarray(gn_b, np.float32)

    nc1 = build_kernel()
    maps1 = _host_inputs(x, w_off, b_off, w_def, b_def, gn_w, gn_b)
    res1 = run_bass_kernel_spmd(nc1, maps1, core_ids=list(range(8)))
    global LAST_RES1, LAST_EXEC_NS
    LAST_RES1 = res1
    if res1.exec_time_ns is not None:
        LAST_EXEC_NS = res1.exec_time_ns

    y = np.zeros((B, O, H, W), np.float32)
    dy = np.zeros((B, 9, H, W), np.float32)
    dx = np.zeros((B, 9, H, W), np.float32)
    A = np.zeros((B, O), np.float32)
    for core in range(8):
        bb, hf = core // 2, core % 2
        r = res1.results[core]
        y[bb, :, hf * HH : (hf + 1) * HH, :] = (
            r["y"].astype(np.float32).reshape(O, HH, W)
        )
        # dy_out partitions are (quarter q, k of 32)
        dyc = r["dy_out"].reshape(4, 32, 2048)[:, 0:9].transpose(1, 0, 2).reshape(9, NPX)
        dxc = r["dx_out"].reshape(4, 32, 2048)[:, 0:9].transpose(1, 0, 2).reshape(9, NPX)
        dy[bb, :, hf * HH : (hf + 1) * HH, :] = dyc.reshape(9, HH, W)
        dx[bb, :, hf * HH : (hf + 1) * HH, :] = dxc.reshape(9, HH, W)
        A[bb] = r["ab_out"][:, 0]

    # exact host patch of |d|>1 sites (clamped on device): invert the leaky
    # relu, add A*(exact - clamped) contribution, re-apply.
    viol = (np.abs(dy) > 1) | (np.abs(dx) > 1)
    for bb, k, h, w in np.argwhere(viol):
        t = _bilin(x[bb], k, h, w, dy[bb, k, h, w], dx[bb, k, h, w])
        c = _bilin(
            x[bb], k, h, w,
            np.clip(dy[bb, k, h, w], -1, 1), np.clip(dx[bb, k, h, w], -1, 1),
        )
        dout = w_def[:, :, k // 3, k % 3] @ (t - c)
        yv = y[bb, :, h, w]
        z = np.where(yv >= 0, yv, yv / NEG) + A[bb] * dout
        y[bb, :, h, w] = np.where(z >= 0, z, NEG * z)
    return y
